# revision 10
# baseline (speedup 1.0000x reference)
"""AttentionReadout (segment softmax attention pooling) on 8 Trainium2 cores.

Math (reference):
    logits = tanh(x @ W1 + b1) @ W2 + b2          # [N, 4]
    attn   = segment_softmax(logits, batch)       # [N, 4]
    xt     = x @ Wt + bt                          # [N, 256] -> [N, 4, 64]
    graph_emb = segment_sum(attn[:, :, None] * xt)  # [1024, 256]
    returns (graph_emb, attn)

Strategy:
  * Host packs nodes into a segment-padded layout: every segment gets L
    (>=512, multiple of 128) node slots, padding rows are zero and carry a
    -30000 logit mask.  1024 segments / 8 cores = 128 whole segments per
    core -> segment reductions are device-local, fixed-shape free-dim
    reductions.
  * Host pre-transposes x per segment to [hidden, L] so every matmul uses
    the weights as natural lhsT ([K, M]) and the activations as rhs
    ([K, N=L]) -- no on-device transposes.
  * All matmuls run as float32r (full fp32 data, 1 cycle/row at N>=256).
  * exp+segment-sum fuse into one scalar-engine activation (accum_out);
    (xt + bt) * attn and the graph sum fuse into one DVE
    scalar_tensor_tensor with accum_out.
  * Outputs: attn in transposed padded layout [nseg, 4, L]; graph
    embeddings as [2, 128, nseg] columns.  Host scatters back.
"""

import numpy as np
from contextlib import ExitStack

import concourse.bass as bass
import concourse.bacc as bacc
import concourse.tile as tile
from concourse import mybir
from concourse.bass_utils import run_bass_kernel_spmd

N_CORES = 8
H = 256
NUM_HEADS = 4
HEAD_DIM = H // NUM_HEADS
B = 1024
NSEG = B // N_CORES  # segments per core
NEG_MASK = -30000.0

F32 = mybir.dt.float32
F32R = mybir.dt.float32r
AX = mybir.AxisListType.X
ALU = mybir.AluOpType
ACT = mybir.ActivationFunctionType

# compiled program cache, keyed by L (pad length per segment)
_programs: dict[int, bass.Bass] = {}

LAST_EXEC_NS = None
LAST_RESULT = None
TRACE = False


def _build_program(L: int, nseg: int = NSEG) -> bass.Bass:
    nc = bacc.Bacc()

    xpt = nc.dram_tensor("xpt", [nseg, 2, 128, L], F32R, kind="ExternalInput")
    maskt = nc.dram_tensor("maskt", [nseg, NUM_HEADS, L], F32, kind="ExternalInput")
    w1 = nc.dram_tensor("w1", [H, H], F32R, kind="ExternalInput")
    w2 = nc.dram_tensor("w2", [H, NUM_HEADS], F32R, kind="ExternalInput")
    wt = nc.dram_tensor("wt", [H, H], F32R, kind="ExternalInput")
    b1d = nc.dram_tensor("b1d", [H, 1], F32, kind="ExternalInput")
    b2d = nc.dram_tensor("b2d", [NUM_HEADS, 1], F32, kind="ExternalInput")
    e4d = nc.dram_tensor("e4d", [2, NUM_HEADS, 128], F32R, kind="ExternalInput")
    btd = nc.dram_tensor("btd", [H, 1], F32, kind="ExternalInput")

    attn_out = nc.dram_tensor(
        "attn_out", [nseg, NUM_HEADS, L], F32R, kind="ExternalOutput"
    )
    g_out = nc.dram_tensor("g_out", [2, 128, nseg], F32, kind="ExternalOutput")

    with tile.TileContext(nc) as tc, ExitStack() as ctx:
        const = ctx.enter_context(tc.tile_pool(name="const", bufs=1))
        sb = ctx.enter_context(tc.tile_pool(name="sb", bufs=2))
        ps = ctx.enter_context(tc.tile_pool(name="ps", bufs=1, space="PSUM"))

        # --- persistent weights / constants ---
        w1s0 = const.tile([128, H], F32R, tag="w1s0", name="w1s0")
        w1s1 = const.tile([128, H], F32R, tag="w1s1", name="w1s1")
        nc.sync.dma_start(w1s0[:], w1[0:128, :])
        nc.sync.dma_start(w1s1[:], w1[128:256, :])
        wts0 = const.tile([128, H], F32R, tag="wts0", name="wts0")
        wts1 = const.tile([128, H], F32R, tag="wts1", name="wts1")
        nc.sync.dma_start(wts0[:], wt[0:128, :])
        nc.sync.dma_start(wts1[:], wt[128:256, :])
        w2s0 = const.tile([128, NUM_HEADS], F32R, tag="w2s0", name="w2s0")
        w2s1 = const.tile([128, NUM_HEADS], F32R, tag="w2s1", name="w2s1")
        nc.sync.dma_start(w2s0[:], w2[0:128, :])
        nc.sync.dma_start(w2s1[:], w2[128:256, :])
        b1s = const.tile([128, 2], F32, tag="b1s", name="b1s")
        nc.sync.dma_start(b1s[:, 0:1], b1d[0:128, :])
        nc.sync.dma_start(b1s[:, 1:2], b1d[128:256, :])
        bts = const.tile([128, 2], F32, tag="bts", name="bts")
        nc.sync.dma_start(bts[:, 0:1], btd[0:128, :])
        nc.sync.dma_start(bts[:, 1:2], btd[128:256, :])
        b2s = const.tile([NUM_HEADS, 1], F32, tag="b2s", name="b2s")
        nc.sync.dma_start(b2s[:], b2d[:, :])

        e4s = const.tile([NUM_HEADS, 2, 128], F32R, tag="e4s", name="e4s")
        nc.sync.dma_start(e4s[:], e4d.transpose([1, 0, 2]))

        # graph embedding accumulators: column s = segment s
        g0 = const.tile([128, nseg], F32, tag="g0", name="g0")
        g1 = const.tile([128, nseg], F32, tag="g1", name="g1")

        w1s = [w1s0, w1s1]
        wts = [wts0, wts1]
        w2s = [w2s0, w2s1]

        for s in range(nseg):
            # ---- load x^T for this segment: two [128, L] chunks ----
            xp0 = sb.tile([128, L], F32R, tag="xp0", bufs=3, name="xp0")
            xp1 = sb.tile([128, L], F32R, tag="xp1", bufs=3, name="xp1")
            nc.sync.dma_start(xp0[:], xpt[s, 0, :, :])
            nc.sync.dma_start(xp1[:], xpt[s, 1, :, :])
            xps = [xp0, xp1]

            mk = sb.tile([NUM_HEADS, L], F32, tag="mk", bufs=3, name="mk")
            nc.sync.dma_start(mk[:], maskt[s, :, :])

            # ---- hT = tanh(W1^T @ x^T + b1): two [128, L] chunks ----
            hs = []
            for j in range(2):
                ph = ps.tile([128, L], F32, tag=f"ph{j}", name=f"ph{j}")
                for i in range(2):
                    nc.tensor.matmul(
                        ph[:],
                        w1s[i][:, j * 128 : (j + 1) * 128],
                        xps[i][:],
                        start=(i == 0),
                        stop=(i == 1),
                    )
                hj = sb.tile([128, L], F32R, tag=f"h{j}", name=f"h{j}")
                nc.scalar.activation(hj[:], ph[:], ACT.Tanh, bias=b1s[:, j : j + 1])
                hs.append(hj)

            # ---- logits^T = W2^T @ hT (+mask): [4, L] ----
            pl = ps.tile([NUM_HEADS, L], F32, tag="pl", bufs=2, name="pl")
            for i in range(2):
                nc.tensor.matmul(
                    pl[:], w2s[i][:], hs[i][:], start=(i == 0), stop=(i == 1)
                )
            lm = sb.tile([NUM_HEADS, L], F32, tag="lm", name="lm")
            nc.vector.tensor_add(lm[:], pl[:], mk[:])

            # ---- segment softmax along free dim ----
            nmax = sb.tile([NUM_HEADS, 1], F32, tag="nmax", name="nmax")
            nc.vector.reduce_max(nmax[:], lm[:], axis=AX, negate=True)
            ebias = sb.tile([NUM_HEADS, 1], F32, tag="ebias", name="ebias")
            nc.vector.tensor_add(ebias[:], nmax[:], b2s[:])
            ev = sb.tile([NUM_HEADS, L], F32, tag="ev", name="ev")
            ssum = sb.tile([NUM_HEADS, 1], F32, tag="ssum", name="ssum")
            nc.scalar.activation(
                ev[:], lm[:], ACT.Exp, bias=ebias[:, 0:1], accum_out=ssum[:]
            )
            rinv = sb.tile([NUM_HEADS, 1], F32, tag="rinv", name="rinv")
            nc.vector.reciprocal(rinv[:], ssum[:])
            at = sb.tile([NUM_HEADS, L], F32R, tag="at", bufs=3, name="at")
            nc.vector.tensor_scalar_mul(at[:], ev[:], rinv[:, 0:1])
            nc.sync.dma_start(attn_out[s, :, :], at[:])

            # ---- replicate attn rows [4,L] -> [128,L] per chunk via PE ----
            reps = []
            for j in range(2):
                rep = ps.tile([128, L], F32, tag=f"rep{j}", name=f"rep{j}")
                nc.tensor.matmul(rep[:], e4s[:, j, :], at[:], start=True, stop=True)
                reps.append(rep)

            # ---- xt^T = Wt^T @ x^T, then (xt + bt) * attn, graph sum ----
            for j in range(2):
                px = ps.tile([128, L], F32, tag=f"px{j}", name=f"px{j}")
                for i in range(2):
                    nc.tensor.matmul(
                        px[:],
                        wts[i][:, j * 128 : (j + 1) * 128],
                        xps[i][:],
                        start=(i == 0),
                        stop=(i == 1),
                    )
                xts = sb.tile([128, L], F32, tag=f"xts{j}", name=f"xts{j}")
                nc.scalar.activation(
                    xts[:], px[:], ACT.Identity, bias=bts[:, j : j + 1]
                )
                wsc = sb.tile([128, L], F32, tag=f"wsc{j}", name=f"wsc{j}")
                gcol = (g0 if j == 0 else g1)[:, s : s + 1]
                nc.vector.scalar_tensor_tensor(
                    wsc[:],
                    xts[:],
                    1.0,
                    reps[j][:],
                    op0=ALU.mult,
                    op1=ALU.mult,
                    accum_out=gcol,
                )

        nc.sync.dma_start(g_out[0, :, :], g0[:])
        nc.sync.dma_start(g_out[1, :, :], g1[:])

    nc.finalize()
    return nc


def kernel(x, batch, W1, b1, W2, b2, Wt, bt):
    global LAST_EXEC_NS, LAST_RESULT

    x = np.ascontiguousarray(np.asarray(x, dtype=np.float32))
    batch = np.asarray(batch)
    N = x.shape[0]

    starts = np.searchsorted(batch, np.arange(B + 1)).astype(np.int64)
    lens = np.diff(starts)
    maxlen = int(lens.max())
    L = max(512, int(np.ceil(maxlen / 128.0)) * 128)

    if L not in _programs:
        _programs[L] = _build_program(L)
    nc = _programs[L]

    # node -> (segment, position-within-segment)
    pos = np.arange(N, dtype=np.int64) - starts[batch]

    # padded, per-segment-transposed x: [B, H, L]
    xp = np.zeros((B, L, H), dtype=np.float32)
    xp[batch, pos] = x
    xpt_all = np.ascontiguousarray(xp.transpose(0, 2, 1))  # [B, H, L]
    del xp

    mask1 = np.full((B, L), NEG_MASK, dtype=np.float32)
    mask1[batch, pos] = 0.0
    mask_all = np.ascontiguousarray(
        np.broadcast_to(mask1[:, None, :], (B, NUM_HEADS, L))
    )

    W1 = np.ascontiguousarray(np.asarray(W1, dtype=np.float32))
    W2 = np.ascontiguousarray(np.asarray(W2, dtype=np.float32))
    Wt = np.ascontiguousarray(np.asarray(Wt, dtype=np.float32))
    b1c = np.ascontiguousarray(np.asarray(b1, dtype=np.float32).reshape(H, 1))
    b2c = np.ascontiguousarray(np.asarray(b2, dtype=np.float32).reshape(NUM_HEADS, 1))
    e4 = np.zeros((2, NUM_HEADS, 128), dtype=np.float32)
    for j in range(2):
        for h in range(2):
            e4[j, 2 * j + h, h * HEAD_DIM : (h + 1) * HEAD_DIM] = 1.0
    btc = np.ascontiguousarray(np.asarray(bt, dtype=np.float32).reshape(H, 1))

    in_maps = []
    for c in range(N_CORES):
        seg_lo = c * NSEG
        xpt_c = np.ascontiguousarray(
            xpt_all[seg_lo : seg_lo + NSEG].reshape(NSEG, 2, 128, L)
        )
        in_maps.append(
            dict(
                xpt=xpt_c,
                maskt=np.ascontiguousarray(mask_all[seg_lo : seg_lo + NSEG]),
                w1=W1,
                w2=W2,
                wt=Wt,
                b1d=b1c,
                b2d=b2c,
                e4d=e4,
                btd=btc,
            )
        )

    res = run_bass_kernel_spmd(nc, in_maps, list(range(N_CORES)), trace=TRACE)
    LAST_RESULT = res
    LAST_EXEC_NS = res.exec_time_ns

    # ---- unpack ----
    graph_emb = np.empty((B, H), dtype=np.float32)
    attn_all = np.empty((B, NUM_HEADS, L), dtype=np.float32)
    for c in range(N_CORES):
        seg_lo = c * NSEG
        g = res.results[c]["g_out"]  # [2, 128, NSEG]
        graph_emb[seg_lo : seg_lo + NSEG] = g.transpose(2, 0, 1).reshape(NSEG, H)
        attn_all[seg_lo : seg_lo + NSEG] = res.results[c]["attn_out"]

    graph_emb[lens == 0] = 0.0
    attn = attn_all[batch, :, pos]  # [N, 4]
    return graph_emb, attn
